# revision 26
# baseline (speedup 1.0000x reference)
"""Bass/Tile TRN2 kernel for nn_Attn (Bahdanau-style attention scores).

Reference computation (B=32, S=2048, H=1024):
    enc    = transpose(encoder_outputs, (1,0,2))            # [B,S,H]
    cat    = concat([hidden[:,None,:] broadcast, enc], -1)  # [B,S,2H]
    energy = tanh(cat @ W.T + b)                            # [B,S,H]
    scores = energy @ v[0]                                  # [B,S]
    attn   = softmax(scores, axis=-1)[:, None, :]           # [B,1,S]

Distribution: data-parallel over batch. 8 cores x 4 batches each.
W/b/v replicated. Host does layout-only prep (slices + transposes, no
arithmetic): enc arrives per-core already k-major ([sc, H, b, s] blocks)
so the device runs no PE transposes at all; W arrives as W.T.

Per-core algorithm (matmuls in float32r via bitcast: ~14-bit effective
mantissa at 1 cycle/row for moving dim >= 256):
    wt_all [128, 8kj, 1024h] <- W2^T DMA'd k-major (f32r bitcast, no copy)
    u      = W1^T.T @ hidden^T + b, via hidT-stationary matmuls
             ([4,512] psum x2) + 8 tiny PE transposes -> u_all [128,8,4]
    per chunk (sc, bi): encT [128, 8, 512] DMA'd directly (pre-transposed)
      T^T[ho]  = sum_kj wt[kj,ho].T @ encT[kj]      (PSUM accum, 8 mm)
      et       = tanh(T^T + u[:,ho,bi])             (ACT, bias column)
      acc     += et * v[ho]                         (DVE fused mul-add)
      pscore[bi,:] += ones-masked partition-sum of acc   (1 matmul/chunk)
    softmax over S on [4, 2048], DMA out.
"""

import numpy as np

B, S, H = 32, 2048, 1024
NCORES = 8
BPC = B // NCORES          # batches per core
SC = 512                   # s-chunk (matmul moving size)
NSC = S // SC              # chunks per batch
KB = H // 128              # 128-blocks along one H
P = 128

_compiled = {}


def _build():
    import concourse.bass as bass
    import concourse.mybir as mybir
    from concourse import bacc, tile, masks

    f32 = mybir.dt.float32
    f32r = mybir.dt.float32r
    Tanh = mybir.ActivationFunctionType.Tanh
    Exp = mybir.ActivationFunctionType.Exp
    Mult = mybir.AluOpType.mult
    Add = mybir.AluOpType.add

    nc = bacc.Bacc("TRN2", target_bir_lowering=False, debug=False,
                   num_devices=NCORES)

    # host supplies pre-transposed layouts (pure layout changes, no math):
    #   enc_t: [NSC, H, BPC, SC]  enc^T per s-chunk (k-major)
    #   wt:    W.T [2H, H]        (k-major)
    #   hidt:  [128, 8, BPC]      hidden.T blocked
    #   biast: [128, 8]           b blocked
    #   vt:    [128, 8]           v blocked
    enc_d = nc.declare_dram_parameter("enc_t", [NSC, H, BPC, SC], f32r,
                                      isOutput=False)
    wt_d = nc.declare_dram_parameter("wt", [2 * H, H], f32r, isOutput=False)
    hidt_d = nc.declare_dram_parameter("hidt", [P, KB, BPC], f32r,
                                       isOutput=False)
    biast_d = nc.declare_dram_parameter("biast", [P, KB], f32, isOutput=False)
    vt_d = nc.declare_dram_parameter("vt", [P, KB], f32, isOutput=False)
    out_d = nc.declare_dram_parameter("attn", [BPC, S], f32, isOutput=True)

    with tile.TileContext(nc) as tc:
        import contextlib
        with contextlib.ExitStack() as ctx:
            const = ctx.enter_context(tc.tile_pool(name="const", bufs=1))
            persist = ctx.enter_context(tc.tile_pool(name="persist", bufs=1))
            wnat = ctx.enter_context(tc.tile_pool(name="wnat", bufs=2))
            encp = ctx.enter_context(tc.tile_pool(name="encp", bufs=4))
            etp = ctx.enter_context(tc.tile_pool(name="etp", bufs=3))
            accp = ctx.enter_context(tc.tile_pool(name="accp", bufs=2))
            ps_m = ctx.enter_context(
                tc.tile_pool(name="ps_m", bufs=6, space="PSUM"))
            ps_s = ctx.enter_context(
                tc.tile_pool(name="ps_s", bufs=2, space="PSUM"))

            # ---------- W2^T: 8 direct k-slice DMAs (f32, bitcast at use) --
            wt_all = persist.tile([P, KB, H], f32r, tag="wt")
            for kj in range(KB):
                nc.sync.dma_start(
                    wt_all[:, kj, :],
                    wt_d[H + kj * P:H + (kj + 1) * P, :])

            # ---------- enc chunk prefetch ----------
            chunks = [(sc, bi) for sc in range(NSC) for bi in range(BPC)]
            PREFETCH = 3
            pending = {}

            def fetch(idx):
                sc, bi = chunks[idx]
                t = encp.tile([P, KB, SC], f32r, tag="enc",
                              name=f"enc{sc}_{bi}")
                nc.gpsimd.dma_start(
                    t[:],
                    enc_d[sc, :, bi, :].rearrange("(a p) s -> p a s", p=P))
                return t

            for idx in range(PREFETCH):
                pending[idx] = fetch(idx)

            # ---------- small constants ----------
            hidT = const.tile([P, KB, BPC], f32r, tag="hidT")
            nc.scalar.dma_start(hidT[:], hidt_d[:])
            biasT = const.tile([P, KB], f32, tag="biasT")
            nc.scalar.dma_start(biasT[:], biast_d[:])
            vT = const.tile([P, KB], f32, tag="vT")
            nc.scalar.dma_start(vT[:], vt_d[:])

            ident = const.tile([P, P], f32, tag="ident")
            masks.make_identity(nc, ident[:])

            # masked-ones stationaries: mask4[:, c, bi] = 1.0 iff c == bi,
            # so matmul with stationary mask4[:, :, bi] puts the partition
            # sum of the moving tile into psum row bi (other rows += 0).
            ones = const.tile([P, 1], f32, tag="ones")
            nc.gpsimd.memset(ones[:], 1.0)
            mask4 = const.tile([P, BPC, BPC], f32r, tag="mask4")
            zt = wnat.tile([P, BPC * BPC], f32, tag="zero", bufs=1)
            nc.gpsimd.memset(zt[:], 0.0)
            nc.vector.tensor_copy(
                mask4[:].rearrange("p a b -> p (a b)"), zt[:])
            for bi in range(BPC):
                nc.vector.tensor_copy(mask4[:, bi, bi:bi + 1], ones[:])

            # ---------- u = W1^T.T @ hidden^T (+ bias) --------------------
            # hidT slices stationary, W1 rows moving -> u_bh [4b, 1024h],
            # then 8 tiny PE transposes -> u_all [128, ho, b] (+ bias).
            pu_a = ps_s.tile([BPC, SC], f32, tag="ps_small")
            pu_b = ps_s.tile([BPC, SC], f32, tag="ps_small")
            for kj in range(KB):
                w1r = wnat.tile([P, H], f32r, tag="w1r")
                nc.scalar.dma_start(w1r[:], wt_d[kj * P:(kj + 1) * P, :])
                nc.tensor.matmul(
                    pu_a[:], hidT[:, kj, :], w1r[:, 0:SC],
                    start=(kj == 0), stop=(kj == KB - 1))
                nc.tensor.matmul(
                    pu_b[:], hidT[:, kj, :], w1r[:, SC:H],
                    start=(kj == 0), stop=(kj == KB - 1))
            u_bh = const.tile([BPC, H], f32, tag="u_bh")
            nc.vector.tensor_copy(u_bh[:, 0:SC], pu_a[:])
            nc.vector.tensor_copy(u_bh[:, SC:H], pu_b[:])

            u_all = const.tile([P, KB, BPC], f32, tag="u")
            for ho in range(KB):
                put = ps_s.tile([P, BPC], f32, tag="ps_small", name=f"put{ho}")
                nc.tensor.transpose(
                    put[:], u_bh[:, ho * P:(ho + 1) * P],
                    ident[0:BPC, 0:BPC])
                nc.vector.tensor_scalar_add(
                    u_all[:, ho, :], put[:], biasT[:, ho:ho + 1])

            # ---------- scores buffer ----------
            scores = persist.tile([BPC, S], f32, tag="scores")
            cmx = const.tile([BPC, NSC], f32, tag="cmx")

            # ---------- main loop ----------
            for idx, (sc, bi) in enumerate(chunks):
                s0 = sc * SC
                encT = pending.pop(idx)
                if idx + PREFETCH < len(chunks):
                    pending[idx + PREFETCH] = fetch(idx + PREFETCH)

                acc = accp.tile([P, SC], f32r, tag="acc",
                                name=f"acc{sc}_{bi}")
                for ho in range(KB):
                    pm = ps_m.tile([P, SC], f32, tag="pm",
                                   name=f"pm{sc}_{bi}_{ho}")
                    for kj in range(KB):
                        nc.tensor.matmul(
                            pm[:],
                            wt_all[:, kj, ho * P:(ho + 1) * P],
                            encT[:, kj, :],
                            start=(kj == 0), stop=(kj == KB - 1))
                    et = etp.tile([P, SC], f32, tag="et",
                                  name=f"et{sc}_{bi}_{ho}")
                    nc.scalar.activation(
                        et[:], pm[:], Tanh,
                        bias=u_all[:, ho, bi:bi + 1], scale=1.0)
                    if ho == 0:
                        nc.vector.tensor_scalar_mul(
                            acc[:], et[:], vT[:, 0:1])
                    else:
                        nc.vector.scalar_tensor_tensor(
                            acc[:], et[:], vT[:, ho:ho + 1], acc[:],
                            op0=Mult, op1=Add)

                if bi == 0:
                    pscore = ps_s.tile([BPC, SC], f32, tag="ps_small",
                                       name=f"pscore{sc}")
                nc.tensor.matmul(
                    pscore[:], mask4[:, :, bi],
                    acc[:],
                    start=(bi == 0), stop=(bi == BPC - 1))
                if bi == BPC - 1:
                    nc.vector.tensor_copy(scores[:, s0:s0 + SC], pscore[:])
                    nc.vector.reduce_max(
                        cmx[:, sc:sc + 1], scores[:, s0:s0 + SC],
                        axis=mybir.AxisListType.X)

            # ---------- softmax over S (4 partitions x 2048) ----------
            mx = const.tile([BPC, 1], f32, tag="mx")
            nc.vector.reduce_max(mx[:], cmx[:], axis=mybir.AxisListType.X)
            nmx = const.tile([BPC, 1], f32, tag="nmx")
            nc.vector.tensor_scalar_mul(nmx[:], mx[:], -1.0)
            ssum = const.tile([BPC, 1], f32, tag="ssum")
            attn_sb = persist.tile([BPC, S], f32, tag="attn")
            nc.scalar.activation(attn_sb[:], scores[:], Exp,
                                 bias=nmx[:], scale=1.0, accum_out=ssum[:])
            rs = const.tile([BPC, 1], f32, tag="rs")
            nc.vector.reciprocal(rs[:], ssum[:])
            nc.vector.tensor_scalar_mul(attn_sb[:], attn_sb[:], rs[:])
            nc.sync.dma_start(out_d[:], attn_sb[:])

    nc.compile()
    return nc


def _get_nc():
    if "nc" not in _compiled:
        _compiled["nc"] = _build()
    return _compiled["nc"]


def _make_in_maps(hidden, encoder_outputs, W, b, v):
    hidden = np.ascontiguousarray(hidden, dtype=np.float32)
    encoder_outputs = np.ascontiguousarray(encoder_outputs, dtype=np.float32)
    W = np.asarray(W, dtype=np.float32)
    b = np.asarray(b, dtype=np.float32).reshape(H)
    v = np.asarray(v, dtype=np.float32).reshape(H)

    # layout-only host prep (replicated across cores)
    wt = np.ascontiguousarray(W.T)                                  # [2H, H]
    biast = np.ascontiguousarray(b.reshape(KB, P).T)                # [128, 8]
    vt = np.ascontiguousarray(v.reshape(KB, P).T)                   # [128, 8]

    in_maps = []
    for c in range(NCORES):
        bs = slice(c * BPC, (c + 1) * BPC)
        hidt = np.ascontiguousarray(
            hidden[bs].T.reshape(KB, P, BPC).transpose(1, 0, 2))    # [128,8,4]
        # enc^T per s-chunk: enc_t[sc, h, b, s] = enc[sc*SC+s, b, h]
        # (pure transpose, done blockwise for cache locality)
        sl = encoder_outputs[:, bs, :]                              # [S,4,H]
        enc_t = np.empty((NSC, H, BPC, SC), np.float32)
        for sc in range(NSC):
            blk = sl[sc * SC:(sc + 1) * SC]                         # [512,4,H]
            for bi in range(BPC):
                enc_t[sc, :, bi, :] = blk[:, bi, :].T
        in_maps.append({
            "enc_t": enc_t,
            "wt": wt,
            "hidt": hidt,
            "biast": biast,
            "vt": vt,
        })
    return in_maps


def kernel(hidden, encoder_outputs, W, b, v):
    from concourse.bass_utils import run_bass_kernel_spmd

    nc = _get_nc()
    in_maps = _make_in_maps(hidden, encoder_outputs, W, b, v)
    res = run_bass_kernel_spmd(nc, in_maps, list(range(NCORES)))
    _compiled["last_result"] = res
    attn = np.concatenate(
        [res.results[c]["attn"] for c in range(NCORES)], axis=0)  # [B, S]
    return attn[:, None, :].astype(np.float32)


if __name__ == "__main__":
    rng = np.random.default_rng(0)
    inputs = {
        "hidden": rng.standard_normal((B, H)).astype(np.float32),
        "encoder_outputs": rng.standard_normal((S, B, H)).astype(np.float32),
        "W": (rng.standard_normal((H, 2 * H)) / np.sqrt(2 * H)).astype(np.float32),
        "b": (rng.standard_normal(H) * 0.01).astype(np.float32),
        "v": rng.standard_normal((1, H)).astype(np.float32),
    }
    out = kernel(**inputs)
    print("out", out.shape, out.dtype, out.sum())


# revision 27
# speedup vs baseline: 1.1853x; 1.1853x over previous
"""Bass/Tile TRN2 kernel for nn_Attn (Bahdanau-style attention scores).

Reference computation (B=32, S=2048, H=1024):
    enc    = transpose(encoder_outputs, (1,0,2))            # [B,S,H]
    cat    = concat([hidden[:,None,:] broadcast, enc], -1)  # [B,S,2H]
    energy = tanh(cat @ W.T + b)                            # [B,S,H]
    scores = energy @ v[0]                                  # [B,S]
    attn   = softmax(scores, axis=-1)[:, None, :]           # [B,1,S]

Distribution: data-parallel over batch. 8 cores x 4 batches each.
W/b/v replicated. Host does layout-only prep (slices + transposes, no
arithmetic): enc arrives per-core already k-major ([sc, H, b, s] blocks)
so the device runs no PE transposes at all; W arrives as W.T.

Per-core algorithm (matmuls in float32r via bitcast: ~14-bit effective
mantissa at 1 cycle/row for moving dim >= 256):
    wt_all [128, 8kj, 1024h] <- W2^T DMA'd k-major (f32r bitcast, no copy)
    u      = W1^T.T @ hidden^T + b, via hidT-stationary matmuls
             ([4,512] psum x2) + 8 tiny PE transposes -> u_all [128,8,4]
    per chunk (sc, bi): encT [128, 8, 512] DMA'd directly (pre-transposed)
      T^T[ho]  = sum_kj wt[kj,ho].T @ encT[kj]      (PSUM accum, 8 mm)
      et       = tanh(T^T + u[:,ho,bi])             (ACT, bias column)
      acc     += et * v[ho]                         (DVE fused mul-add)
      pscore[bi,:] += ones-masked partition-sum of acc   (1 matmul/chunk)
    softmax over S on [4, 2048], DMA out.
"""

import numpy as np

B, S, H = 32, 2048, 1024
NCORES = 8
BPC = B // NCORES          # batches per core
SC = 512                   # s-chunk (matmul moving size)
NSC = S // SC              # chunks per batch
KB = H // 128              # 128-blocks along one H
P = 128

_compiled = {}


def _build():
    import concourse.bass as bass
    import concourse.mybir as mybir
    from concourse import bacc, tile, masks

    f32 = mybir.dt.float32
    f32r = mybir.dt.float32r
    Tanh = mybir.ActivationFunctionType.Tanh
    Exp = mybir.ActivationFunctionType.Exp
    Mult = mybir.AluOpType.mult
    Add = mybir.AluOpType.add

    nc = bacc.Bacc("TRN2", target_bir_lowering=False, debug=False,
                   num_devices=NCORES)

    # host supplies pre-transposed layouts (pure layout changes, no math):
    #   enc_t: [NSC, H, BPC, SC]  enc^T per s-chunk (k-major)
    #   wt:    W.T [2H, H]        (k-major)
    #   hidt:  [128, 8, BPC]      hidden.T blocked
    #   biast: [128, 8]           b blocked
    #   vt:    [128, 8]           v blocked
    enc_d = nc.declare_dram_parameter("enc_t", [NSC, H, BPC, SC], f32r,
                                      isOutput=False)
    wt_d = nc.declare_dram_parameter("wt", [2 * H, H], f32r, isOutput=False)
    hidt_d = nc.declare_dram_parameter("hidt", [P, KB, BPC], f32r,
                                       isOutput=False)
    biast_d = nc.declare_dram_parameter("biast", [P, KB], f32, isOutput=False)
    vt_d = nc.declare_dram_parameter("vt", [P, KB], f32, isOutput=False)
    out_d = nc.declare_dram_parameter("attn", [BPC, S], f32, isOutput=True)

    with tile.TileContext(nc) as tc:
        import contextlib
        with contextlib.ExitStack() as ctx:
            const = ctx.enter_context(tc.tile_pool(name="const", bufs=1))
            persist = ctx.enter_context(tc.tile_pool(name="persist", bufs=1))
            wnat = ctx.enter_context(tc.tile_pool(name="wnat", bufs=2))
            encp = ctx.enter_context(tc.tile_pool(name="encp", bufs=4))
            etp = ctx.enter_context(tc.tile_pool(name="etp", bufs=3))
            accp = ctx.enter_context(tc.tile_pool(name="accp", bufs=2))
            ps_m = ctx.enter_context(
                tc.tile_pool(name="ps_m", bufs=5, space="PSUM"))
            ps_s = ctx.enter_context(
                tc.tile_pool(name="ps_s", bufs=3, space="PSUM"))

            # ---------- W2^T: 8 direct k-slice DMAs (f32, bitcast at use) --
            wt_all = persist.tile([P, KB, H], f32r, tag="wt")
            for kj in range(KB):
                nc.sync.dma_start(
                    wt_all[:, kj, :],
                    wt_d[H + kj * P:H + (kj + 1) * P, :])

            # ---------- enc chunk prefetch ----------
            chunks = [(sc, bi) for sc in range(NSC) for bi in range(BPC)]
            PREFETCH = 3
            pending = {}

            def fetch(idx):
                sc, bi = chunks[idx]
                t = encp.tile([P, KB, SC], f32r, tag="enc",
                              name=f"enc{sc}_{bi}")
                nc.gpsimd.dma_start(
                    t[:],
                    enc_d[sc, :, bi, :].rearrange("(a p) s -> p a s", p=P))
                return t

            for idx in range(PREFETCH):
                pending[idx] = fetch(idx)

            # ---------- small constants ----------
            hidT = const.tile([P, KB, BPC], f32r, tag="hidT")
            nc.scalar.dma_start(hidT[:], hidt_d[:])
            biasT = const.tile([P, KB], f32, tag="biasT")
            nc.scalar.dma_start(biasT[:], biast_d[:])
            vT = const.tile([P, KB], f32, tag="vT")
            nc.scalar.dma_start(vT[:], vt_d[:])

            ident = const.tile([P, P], f32, tag="ident")
            masks.make_identity(nc, ident[:])

            # masked-ones stationaries: mask4[:, c, bi] = 1.0 iff c == bi,
            # so matmul with stationary mask4[:, :, bi] puts the partition
            # sum of the moving tile into psum row bi (other rows += 0).
            ones = const.tile([P, 1], f32, tag="ones")
            nc.gpsimd.memset(ones[:], 1.0)
            mask4 = const.tile([P, BPC, BPC], f32r, tag="mask4")
            zt = wnat.tile([P, BPC * BPC], f32, tag="zero", bufs=1)
            nc.gpsimd.memset(zt[:], 0.0)
            nc.vector.tensor_copy(
                mask4[:].rearrange("p a b -> p (a b)"), zt[:])
            for bi in range(BPC):
                nc.vector.tensor_copy(mask4[:, bi, bi:bi + 1], ones[:])

            # ---------- u = W1^T.T @ hidden^T (+ bias) --------------------
            # hidT slices stationary, W1 rows moving -> u_bh [4b, 1024h],
            # then 8 tiny PE transposes -> u_all [128, ho, b] (+ bias).
            pu_a = ps_s.tile([BPC, SC], f32, tag="ps_small")
            pu_b = ps_s.tile([BPC, SC], f32, tag="ps_small")
            for kj in range(KB):
                w1r = wnat.tile([P, H], f32r, tag="w1r")
                nc.scalar.dma_start(w1r[:], wt_d[kj * P:(kj + 1) * P, :])
                nc.tensor.matmul(
                    pu_a[:], hidT[:, kj, :], w1r[:, 0:SC],
                    start=(kj == 0), stop=(kj == KB - 1))
                nc.tensor.matmul(
                    pu_b[:], hidT[:, kj, :], w1r[:, SC:H],
                    start=(kj == 0), stop=(kj == KB - 1))
            u_bh = const.tile([BPC, H], f32, tag="u_bh")
            nc.vector.tensor_copy(u_bh[:, 0:SC], pu_a[:])
            nc.vector.tensor_copy(u_bh[:, SC:H], pu_b[:])

            u_all = const.tile([P, KB, BPC], f32, tag="u")
            for ho in range(KB):
                put = ps_s.tile([P, BPC], f32, tag="ps_small", name=f"put{ho}")
                nc.tensor.transpose(
                    put[:], u_bh[:, ho * P:(ho + 1) * P],
                    ident[0:BPC, 0:BPC])
                nc.vector.tensor_scalar_add(
                    u_all[:, ho, :], put[:], biasT[:, ho:ho + 1])

            # ---------- scores buffer ----------
            scores = persist.tile([BPC, S], f32, tag="scores")
            cmx = const.tile([BPC, NSC], f32, tag="cmx")

            # ---------- main loop ----------
            for idx, (sc, bi) in enumerate(chunks):
                s0 = sc * SC
                encT = pending.pop(idx)
                if idx + PREFETCH < len(chunks):
                    pending[idx + PREFETCH] = fetch(idx + PREFETCH)

                acc = accp.tile([P, SC], f32r, tag="acc",
                                name=f"acc{sc}_{bi}")
                for ho in range(KB):
                    pm = ps_m.tile([P, SC], f32, tag="pm",
                                   name=f"pm{sc}_{bi}_{ho}")
                    for kj in range(KB):
                        nc.tensor.matmul(
                            pm[:],
                            wt_all[:, kj, ho * P:(ho + 1) * P],
                            encT[:, kj, :],
                            start=(kj == 0), stop=(kj == KB - 1))
                    et = etp.tile([P, SC], f32, tag="et",
                                  name=f"et{sc}_{bi}_{ho}")
                    nc.scalar.activation(
                        et[:], pm[:], Tanh,
                        bias=u_all[:, ho, bi:bi + 1], scale=1.0)
                    if ho == 0:
                        nc.vector.tensor_scalar_mul(
                            acc[:], et[:], vT[:, 0:1])
                    else:
                        nc.vector.scalar_tensor_tensor(
                            acc[:], et[:], vT[:, ho:ho + 1], acc[:],
                            op0=Mult, op1=Add)

                if bi == 0:
                    pscore = ps_s.tile([BPC, SC], f32, tag="ps_small",
                                       name=f"pscore{sc}")
                nc.tensor.matmul(
                    pscore[:], mask4[:, :, bi],
                    acc[:],
                    start=(bi == 0), stop=(bi == BPC - 1))
                if bi == BPC - 1:
                    nc.vector.tensor_copy(scores[:, s0:s0 + SC], pscore[:])
                    nc.vector.reduce_max(
                        cmx[:, sc:sc + 1], scores[:, s0:s0 + SC],
                        axis=mybir.AxisListType.X)

            # ---------- softmax over S (4 partitions x 2048) ----------
            mx = const.tile([BPC, 1], f32, tag="mx")
            nc.vector.reduce_max(mx[:], cmx[:], axis=mybir.AxisListType.X)
            nmx = const.tile([BPC, 1], f32, tag="nmx")
            nc.vector.tensor_scalar_mul(nmx[:], mx[:], -1.0)
            ssum = const.tile([BPC, 1], f32, tag="ssum")
            attn_sb = persist.tile([BPC, S], f32, tag="attn")
            nc.scalar.activation(attn_sb[:], scores[:], Exp,
                                 bias=nmx[:], scale=1.0, accum_out=ssum[:])
            rs = const.tile([BPC, 1], f32, tag="rs")
            nc.vector.reciprocal(rs[:], ssum[:])
            nc.vector.tensor_scalar_mul(attn_sb[:], attn_sb[:], rs[:])
            nc.sync.dma_start(out_d[:], attn_sb[:])

    nc.compile()
    return nc


def _get_nc():
    if "nc" not in _compiled:
        _compiled["nc"] = _build()
    return _compiled["nc"]


def _make_in_maps(hidden, encoder_outputs, W, b, v):
    hidden = np.ascontiguousarray(hidden, dtype=np.float32)
    encoder_outputs = np.ascontiguousarray(encoder_outputs, dtype=np.float32)
    W = np.asarray(W, dtype=np.float32)
    b = np.asarray(b, dtype=np.float32).reshape(H)
    v = np.asarray(v, dtype=np.float32).reshape(H)

    # layout-only host prep (replicated across cores)
    wt = np.ascontiguousarray(W.T)                                  # [2H, H]
    biast = np.ascontiguousarray(b.reshape(KB, P).T)                # [128, 8]
    vt = np.ascontiguousarray(v.reshape(KB, P).T)                   # [128, 8]

    in_maps = []
    for c in range(NCORES):
        bs = slice(c * BPC, (c + 1) * BPC)
        hidt = np.ascontiguousarray(
            hidden[bs].T.reshape(KB, P, BPC).transpose(1, 0, 2))    # [128,8,4]
        # enc^T per s-chunk: enc_t[sc, h, b, s] = enc[sc*SC+s, b, h]
        # (pure transpose, done blockwise for cache locality)
        sl = encoder_outputs[:, bs, :]                              # [S,4,H]
        enc_t = np.empty((NSC, H, BPC, SC), np.float32)
        for sc in range(NSC):
            blk = sl[sc * SC:(sc + 1) * SC]                         # [512,4,H]
            for bi in range(BPC):
                enc_t[sc, :, bi, :] = blk[:, bi, :].T
        in_maps.append({
            "enc_t": enc_t,
            "wt": wt,
            "hidt": hidt,
            "biast": biast,
            "vt": vt,
        })
    return in_maps


def kernel(hidden, encoder_outputs, W, b, v):
    from concourse.bass_utils import run_bass_kernel_spmd

    nc = _get_nc()
    in_maps = _make_in_maps(hidden, encoder_outputs, W, b, v)
    res = run_bass_kernel_spmd(nc, in_maps, list(range(NCORES)))
    _compiled["last_result"] = res
    attn = np.concatenate(
        [res.results[c]["attn"] for c in range(NCORES)], axis=0)  # [B, S]
    return attn[:, None, :].astype(np.float32)


if __name__ == "__main__":
    rng = np.random.default_rng(0)
    inputs = {
        "hidden": rng.standard_normal((B, H)).astype(np.float32),
        "encoder_outputs": rng.standard_normal((S, B, H)).astype(np.float32),
        "W": (rng.standard_normal((H, 2 * H)) / np.sqrt(2 * H)).astype(np.float32),
        "b": (rng.standard_normal(H) * 0.01).astype(np.float32),
        "v": rng.standard_normal((1, H)).astype(np.float32),
    }
    out = kernel(**inputs)
    print("out", out.shape, out.dtype, out.sum())
